# revision 5
# baseline (speedup 1.0000x reference)
"""RBF (Gaussian) kernel Gram matrix on 8 Trainium2 NeuronCores.

out[i, j] = exp(-gamma * ||x_i - y_j||^2),  x, y: [8192, 256] fp32.

Strategy (per the data-parallel-over-rows sharding):
  - Rows of x are sharded across the 8 cores (1024 rows each); y replicated.
  - Each core computes its [1024, 8192] stripe:
      psum = x_shard @ y.T - 0.5*||y||^2   (the -0.5*||y||^2 row is folded in
                                            as a k=1 matmul with a ones vector)
      out  = exp(2*gamma * psum + bias)    (bias = -gamma*||x||^2 per-partition,
                                            applied by the ACT engine for free)
    which equals exp(-gamma*(||x||^2 + ||y||^2 - 2*x.y)) exactly.
  - Host prep: transposed copies of x/y (contraction dim on partitions) and the
    row norms. PE does the GEMM, ACT does the exp straight out of PSUM, DMA
    streams the 33.5MB/core result to DRAM. All stages pipelined by Tile.
"""

import numpy as np

GAMMA = 0.005
FULL_N = 8192
D = 256
N_CORES = 8
M_SHARD = FULL_N // N_CORES  # 1024 rows of x per core
P = 128
M_TILES = M_SHARD // P  # 8
GROUP = 2048  # columns of output produced per PSUM fill (4 banks)
BANK = 512  # fp32 columns per PSUM bank (one matmul's max free dim)
N_GROUPS = FULL_N // GROUP  # 4

_cache = {}


def _split_sync_waits(nc, maxw=1):
    """walrus codegen rejects instructions carrying more than ~2 sync waits
    ("Too many sync wait commands"). Tile can attach many (e.g. the tail
    drain waits on every semaphore; a matmul can wait on several DMA lanes).
    Hoist the excess onto wait-only EventSemaphore instructions inserted
    just before the offender on the same engine (engines execute their
    instructions in block order, so all waits still precede the op)."""
    import concourse.mybir as mybir

    n_new = 0
    for fn in nc.m.functions:
        for bb in fn.blocks:
            insts = bb.instructions
            if not any(
                i.sync_info is not None and len(i.sync_info.on_wait) > maxw
                for i in insts
            ):
                continue
            new = []
            for inst in insts:
                si = inst.sync_info
                if si is not None and len(si.on_wait) > maxw:
                    waits = list(si.on_wait)
                    for i in range(0, len(waits) - maxw, maxw):
                        ev = mybir.InstEventSemaphore(
                            name=f"wsplit_{n_new}", ins=[], outs=[]
                        )
                        n_new += 1
                        ev.engine = inst.engine
                        ev.sync_info = mybir.SyncInfo(
                            on_wait=waits[i : i + maxw], on_update=[]
                        )
                        new.append(ev)
                    si.on_wait = waits[len(waits) - maxw :]
                new.append(inst)
            bb.instructions = new


def _build():
    import concourse.bass as bass
    import concourse.mybir as mybir
    import concourse.tile as tile

    f32 = mybir.dt.float32
    nc = bass.Bass("TRN2", target_bir_lowering=False, debug=False)
    xt = nc.dram_tensor("xt", [D, M_SHARD], f32, kind="ExternalInput").ap()
    yt = nc.dram_tensor("yt", [D, FULL_N], f32, kind="ExternalInput").ap()
    x2 = nc.dram_tensor("x2", [P, M_TILES], f32, kind="ExternalInput").ap()
    y2 = nc.dram_tensor("y2", [1, FULL_N], f32, kind="ExternalInput").ap()
    out = nc.dram_tensor("out", [M_SHARD, FULL_N], f32, kind="ExternalOutput").ap()

    with tile.TileContext(nc) as tc:
        with (
            tc.tile_pool(name="const", bufs=1) as cpool,
            tc.tile_pool(name="outp", bufs=3) as opool,
            tc.tile_pool(name="psum", bufs=2, space="PSUM") as ppool,
        ):
            # y.T resident in SBUF: two k-tiles of [128, 8192] (32KB/partition each).
            yt0 = cpool.tile([P, FULL_N], f32, tag="yt0")
            yt1 = cpool.tile([P, FULL_N], f32, tag="yt1")
            for g in range(N_GROUPS):
                sl = slice(g * GROUP, (g + 1) * GROUP)
                nc.sync.dma_start(out=yt0[:, sl], in_=yt[0:P, sl])
                nc.sync.dma_start(out=yt1[:, sl], in_=yt[P : 2 * P, sl])
            xt0 = cpool.tile([P, M_SHARD], f32, tag="xt0")
            xt1 = cpool.tile([P, M_SHARD], f32, tag="xt1")
            nc.sync.dma_start(out=xt0, in_=xt[0:P, :])
            nc.sync.dma_start(out=xt1, in_=xt[P : 2 * P, :])
            x2sb = cpool.tile([P, M_TILES], f32, tag="x2")
            nc.sync.dma_start(out=x2sb, in_=x2)
            y2sb = cpool.tile([1, FULL_N], f32, tag="y2")
            nc.sync.dma_start(out=y2sb, in_=y2)
            ones = cpool.tile([1, P], f32, tag="ones")
            nc.any.memset(ones, 1.0)

            for t in range(M_TILES):
                msl = slice(t * P, (t + 1) * P)
                for g in range(N_GROUPS):
                    ps = ppool.tile([P, GROUP], f32, tag="ps")
                    for b in range(GROUP // BANK):
                        nsl = slice(g * GROUP + b * BANK, g * GROUP + (b + 1) * BANK)
                        bsl = slice(b * BANK, (b + 1) * BANK)
                        nc.tensor.matmul(
                            ps[:, bsl], xt0[:, msl], yt0[:, nsl],
                            start=True, stop=False,
                        )
                        nc.tensor.matmul(
                            ps[:, bsl], xt1[:, msl], yt1[:, nsl],
                            start=False, stop=False,
                        )
                        # += 1 * (-||y_j||^2 / 2): completes x.y - y2/2
                        nc.tensor.matmul(
                            ps[:, bsl], ones, y2sb[:, nsl],
                            start=False, stop=True,
                        )
                    ot = opool.tile([P, GROUP], f32, tag="ot")
                    # exp(2g*(x.y - y2/2) - g*x2) = exp(-g*(x2 + y2 - 2 x.y))
                    nc.scalar.activation(
                        ot, ps, mybir.ActivationFunctionType.Exp,
                        bias=x2sb[:, t : t + 1], scale=2.0 * GAMMA,
                    )
                    nc.sync.dma_start(
                        out=out[msl, g * GROUP : (g + 1) * GROUP], in_=ot
                    )

    _split_sync_waits(nc)
    return nc


def kernel(x: np.ndarray, y: np.ndarray) -> np.ndarray:
    from concourse import bass_utils

    x = np.asarray(x, dtype=np.float32)
    y = np.asarray(y, dtype=np.float32)

    if "nc" not in _cache:
        _cache["nc"] = _build()
    nc = _cache["nc"]

    yt = np.ascontiguousarray(y.T)  # [256, 8192]
    xt_full = np.ascontiguousarray(x.T)  # [256, 8192]
    x2 = np.sum(x * x, axis=1)  # [8192]
    y2row = (-0.5 * np.sum(y * y, axis=1)).reshape(1, FULL_N).astype(np.float32)

    in_maps = []
    for c in range(N_CORES):
        cols = slice(c * M_SHARD, (c + 1) * M_SHARD)
        x2c = (-GAMMA * x2[cols]).astype(np.float32)
        in_maps.append(
            {
                "xt": np.ascontiguousarray(xt_full[:, cols]),
                "yt": yt,
                "x2": np.ascontiguousarray(x2c.reshape(M_TILES, P).T),
                "y2": y2row,
            }
        )

    res = bass_utils.run_bass_kernel_spmd(
        nc, in_maps, core_ids=list(range(N_CORES))
    )
    _cache["last_result"] = res
    return np.concatenate([res.results[c]["out"] for c in range(N_CORES)], axis=0)


# revision 8
# speedup vs baseline: 1.9556x; 1.9556x over previous
"""RBF (Gaussian) kernel Gram matrix on 8 Trainium2 NeuronCores.

out[i, j] = exp(-gamma * ||x_i - y_j||^2),  x, y: [8192, 256] fp32.

Strategy (per the data-parallel-over-rows sharding):
  - Rows of x are sharded across the 8 cores (1024 rows each); y replicated.
  - Each core computes its [1024, 8192] stripe:
      psum = x_shard @ y.T - 0.5*||y||^2   (the -0.5*||y||^2 row is folded in
                                            as a k=1 matmul with a ones vector)
      out  = exp(2*gamma * psum + bias)    (bias = -gamma*||x||^2 per-partition,
                                            applied by the ACT engine for free)
    which equals exp(-gamma*(||x||^2 + ||y||^2 - 2*x.y)) exactly.
  - Host prep: transposed copies of x/y (contraction dim on partitions) and the
    row norms. PE does the GEMM, ACT does the exp straight out of PSUM, DMA
    streams the 33.5MB/core result to DRAM. All stages pipelined by Tile.
"""

import numpy as np

GAMMA = 0.005
FULL_N = 8192
D = 256
N_CORES = 8
M_SHARD = FULL_N // N_CORES  # 1024 rows of x per core
P = 128
M_TILES = M_SHARD // P  # 8
GROUP = 2048  # columns of output produced per PSUM fill (4 banks)
BANK = 512  # fp32 columns per PSUM bank (one matmul's max free dim)
N_GROUPS = FULL_N // GROUP  # 4

_cache = {}


def _split_sync_waits(nc, maxw=1):
    """walrus codegen rejects instructions carrying more than ~2 sync waits
    ("Too many sync wait commands"). Tile can attach many (e.g. the tail
    drain waits on every semaphore; a matmul can wait on several DMA lanes).
    Hoist the excess onto wait-only EventSemaphore instructions inserted
    just before the offender on the same engine (engines execute their
    instructions in block order, so all waits still precede the op)."""
    import concourse.mybir as mybir

    n_new = 0
    for fn in nc.m.functions:
        for bb in fn.blocks:
            insts = bb.instructions
            if not any(
                i.sync_info is not None and len(i.sync_info.on_wait) > maxw
                for i in insts
            ):
                continue
            new = []
            for inst in insts:
                si = inst.sync_info
                if si is not None and len(si.on_wait) > maxw:
                    waits = list(si.on_wait)
                    for i in range(0, len(waits) - maxw, maxw):
                        ev = mybir.InstEventSemaphore(
                            name=f"wsplit_{n_new}", ins=[], outs=[]
                        )
                        n_new += 1
                        ev.engine = inst.engine
                        ev.sync_info = mybir.SyncInfo(
                            on_wait=waits[i : i + maxw], on_update=[]
                        )
                        new.append(ev)
                    si.on_wait = waits[len(waits) - maxw :]
                new.append(inst)
            bb.instructions = new


def _build():
    import concourse.bass as bass
    import concourse.mybir as mybir
    import concourse.tile as tile

    f32 = mybir.dt.float32
    f16 = mybir.dt.float16
    nc = bass.Bass("TRN2", target_bir_lowering=False, debug=False)
    xt = nc.dram_tensor("xt", [D, M_SHARD], f16, kind="ExternalInput").ap()
    yt = nc.dram_tensor("yt", [D, FULL_N], f16, kind="ExternalInput").ap()
    x2 = nc.dram_tensor("x2", [P, M_TILES], f32, kind="ExternalInput").ap()
    # -||y||^2/2 as an fp16 hi/lo pair (row1 = residual) so the fold loses
    # nothing: fp16(a) + fp16(a - fp16(a)) carries ~2^-21 relative error.
    y2 = nc.dram_tensor("y2", [2, FULL_N], f16, kind="ExternalInput").ap()
    out = nc.dram_tensor("out", [M_SHARD, FULL_N], f32, kind="ExternalOutput").ap()

    with tile.TileContext(nc) as tc:
        with (
            tc.tile_pool(name="const", bufs=1) as cpool,
            tc.tile_pool(name="outp", bufs=3) as opool,
            tc.tile_pool(name="psum", bufs=2, space="PSUM") as ppool,
        ):
            # Small inputs first so the first matmul only gates on its own
            # y.T chunk, not the whole 4MB load.
            xt0 = cpool.tile([P, M_SHARD], f16, tag="xt0")
            xt1 = cpool.tile([P, M_SHARD], f16, tag="xt1")
            nc.sync.dma_start(out=xt0, in_=xt[0:P, :])
            nc.sync.dma_start(out=xt1, in_=xt[P : 2 * P, :])
            x2sb = cpool.tile([P, M_TILES], f32, tag="x2")
            nc.sync.dma_start(out=x2sb, in_=x2)
            y2sb = cpool.tile([2, FULL_N], f16, tag="y2")
            nc.sync.dma_start(out=y2sb, in_=y2)
            ones = cpool.tile([2, P], f16, tag="ones")
            nc.any.memset(ones, 1.0)
            # y.T resident in SBUF: two k-tiles of [128, 8192] fp16.
            yt0 = cpool.tile([P, FULL_N], f16, tag="yt0")
            yt1 = cpool.tile([P, FULL_N], f16, tag="yt1")
            for g in range(N_GROUPS):
                sl = slice(g * GROUP, (g + 1) * GROUP)
                nc.sync.dma_start(out=yt0[:, sl], in_=yt[0:P, sl])
                nc.sync.dma_start(out=yt1[:, sl], in_=yt[P : 2 * P, sl])

            for t in range(M_TILES):
                msl = slice(t * P, (t + 1) * P)
                for g in range(N_GROUPS):
                    ps = ppool.tile([P, GROUP], f32, tag="ps")
                    for b in range(GROUP // BANK):
                        nsl = slice(g * GROUP + b * BANK, g * GROUP + (b + 1) * BANK)
                        bsl = slice(b * BANK, (b + 1) * BANK)
                        nc.tensor.matmul(
                            ps[:, bsl], xt0[:, msl], yt0[:, nsl],
                            start=True, stop=False,
                        )
                        nc.tensor.matmul(
                            ps[:, bsl], xt1[:, msl], yt1[:, nsl],
                            start=False, stop=False,
                        )
                        # += 1 * (-||y_j||^2 / 2) (hi+lo): completes x.y - y2/2
                        nc.tensor.matmul(
                            ps[:, bsl], ones, y2sb[:, nsl],
                            start=False, stop=True,
                        )  # k=2: ones[2,128].T @ y2pair[2,512]
                    ot = opool.tile([P, GROUP], f32, tag="ot")
                    # exp(2g*(x.y - y2/2) - g*x2) = exp(-g*(x2 + y2 - 2 x.y))
                    nc.scalar.activation(
                        ot, ps, mybir.ActivationFunctionType.Exp,
                        bias=x2sb[:, t : t + 1], scale=2.0 * GAMMA,
                    )
                    nc.sync.dma_start(
                        out=out[msl, g * GROUP : (g + 1) * GROUP], in_=ot
                    )

    _split_sync_waits(nc)
    return nc


def kernel(x: np.ndarray, y: np.ndarray) -> np.ndarray:
    from concourse import bass_utils

    x = np.asarray(x, dtype=np.float32)
    y = np.asarray(y, dtype=np.float32)

    if "nc" not in _cache:
        _cache["nc"] = _build()
    nc = _cache["nc"]

    yt = np.ascontiguousarray(y.T.astype(np.float16))  # [256, 8192]
    xt_full = x.T.astype(np.float16)  # [256, 8192]
    x2 = np.sum(x * x, axis=1)  # [8192]
    y2neg = -0.5 * np.sum(y.astype(np.float64) * y.astype(np.float64), axis=1)
    y2hi = y2neg.astype(np.float16)
    y2lo = (y2neg - y2hi.astype(np.float64)).astype(np.float16)
    y2pair = np.ascontiguousarray(np.stack([y2hi, y2lo], axis=0))  # [2, 8192]

    in_maps = []
    for c in range(N_CORES):
        cols = slice(c * M_SHARD, (c + 1) * M_SHARD)
        x2c = (-GAMMA * x2[cols]).astype(np.float32)
        in_maps.append(
            {
                "xt": np.ascontiguousarray(xt_full[:, cols]),
                "yt": yt,
                "x2": np.ascontiguousarray(x2c.reshape(M_TILES, P).T),
                "y2": y2pair,
            }
        )

    res = bass_utils.run_bass_kernel_spmd(
        nc, in_maps, core_ids=list(range(N_CORES))
    )
    _cache["last_result"] = res
    return np.concatenate([res.results[c]["out"] for c in range(N_CORES)], axis=0)


# revision 11
# speedup vs baseline: 1.9569x; 1.0007x over previous
"""RBF (Gaussian) kernel Gram matrix on 8 Trainium2 NeuronCores.

out[i, j] = exp(-gamma * ||x_i - y_j||^2),  x, y: [8192, 256] fp32.

Strategy (per the data-parallel-over-rows sharding):
  - Rows of x are sharded across the 8 cores (1024 rows each); y replicated.
  - Each core computes its [1024, 8192] stripe:
      psum = x_shard @ y.T - 0.5*||y||^2   (the -0.5*||y||^2 row is folded in
                                            as a k=1 matmul with a ones vector)
      out  = exp(2*gamma * psum + bias)    (bias = -gamma*||x||^2 per-partition,
                                            applied by the ACT engine for free)
    which equals exp(-gamma*(||x||^2 + ||y||^2 - 2*x.y)) exactly.
  - Host prep: transposed copies of x/y (contraction dim on partitions) and the
    row norms. PE does the GEMM, ACT does the exp straight out of PSUM, DMA
    streams the 33.5MB/core result to DRAM. All stages pipelined by Tile.
"""

import numpy as np

GAMMA = 0.005
FULL_N = 8192
D = 256
N_CORES = 8
M_SHARD = FULL_N // N_CORES  # 1024 rows of x per core
P = 128
M_TILES = M_SHARD // P  # 8
GROUP = 2048  # columns of output produced per PSUM fill (4 banks)
BANK = 512  # fp32 columns per PSUM bank (one matmul's max free dim)
N_GROUPS = FULL_N // GROUP  # 4

_cache = {}


def _split_sync_waits(nc, maxw=1):
    """walrus codegen rejects instructions carrying more than ~2 sync waits
    ("Too many sync wait commands"). Tile can attach many (e.g. the tail
    drain waits on every semaphore; a matmul can wait on several DMA lanes).
    Hoist the excess onto wait-only EventSemaphore instructions inserted
    just before the offender on the same engine (engines execute their
    instructions in block order, so all waits still precede the op)."""
    import concourse.mybir as mybir

    n_new = 0
    for fn in nc.m.functions:
        for bb in fn.blocks:
            insts = bb.instructions
            if not any(
                i.sync_info is not None and len(i.sync_info.on_wait) > maxw
                for i in insts
            ):
                continue
            new = []
            for inst in insts:
                si = inst.sync_info
                if si is not None and len(si.on_wait) > maxw:
                    waits = list(si.on_wait)
                    for i in range(0, len(waits) - maxw, maxw):
                        ev = mybir.InstEventSemaphore(
                            name=f"wsplit_{n_new}", ins=[], outs=[]
                        )
                        n_new += 1
                        ev.engine = inst.engine
                        ev.sync_info = mybir.SyncInfo(
                            on_wait=waits[i : i + maxw], on_update=[]
                        )
                        new.append(ev)
                    si.on_wait = waits[len(waits) - maxw :]
                new.append(inst)
            bb.instructions = new


def _build():
    import concourse.bass as bass
    import concourse.mybir as mybir
    import concourse.tile as tile

    f32 = mybir.dt.float32
    bf16 = mybir.dt.bfloat16
    nc = bass.Bass("TRN2", target_bir_lowering=False, debug=False)
    xt = nc.dram_tensor("xt", [D, M_SHARD], bf16, kind="ExternalInput").ap()
    yt = nc.dram_tensor("yt", [D, FULL_N], bf16, kind="ExternalInput").ap()
    x2 = nc.dram_tensor("x2", [P, M_TILES], f32, kind="ExternalInput").ap()
    # -||y||^2/2 as a bf16 hi/lo pair (row1 = residual) so the fold loses
    # nothing: bf16(a) + bf16(a - bf16(a)) carries ~2^-18 relative error.
    y2 = nc.dram_tensor("y2", [2, FULL_N], bf16, kind="ExternalInput").ap()
    out = nc.dram_tensor("out", [M_SHARD, FULL_N], f32, kind="ExternalOutput").ap()

    with tile.TileContext(nc) as tc:
        with (
            tc.tile_pool(name="const", bufs=1) as cpool,
            tc.tile_pool(name="outp", bufs=3) as opool,
            tc.tile_pool(name="psum", bufs=2, space="PSUM") as ppool,
        ):
            # Small inputs first so the first matmul only gates on its own
            # y.T chunk, not the whole 4MB load.
            xt0 = cpool.tile([P, M_SHARD], bf16, tag="xt0")
            xt1 = cpool.tile([P, M_SHARD], bf16, tag="xt1")
            nc.sync.dma_start(out=xt0, in_=xt[0:P, :])
            nc.sync.dma_start(out=xt1, in_=xt[P : 2 * P, :])
            x2sb = cpool.tile([P, M_TILES], f32, tag="x2")
            nc.sync.dma_start(out=x2sb, in_=x2)
            y2sb = cpool.tile([2, FULL_N], bf16, tag="y2")
            nc.sync.dma_start(out=y2sb, in_=y2)
            ones = cpool.tile([2, P], bf16, tag="ones")
            nc.any.memset(ones, 1.0)
            # y.T resident in SBUF: two k-tiles of [128, 8192] bf16.
            yt0 = cpool.tile([P, FULL_N], bf16, tag="yt0")
            yt1 = cpool.tile([P, FULL_N], bf16, tag="yt1")
            for g in range(N_GROUPS):
                sl = slice(g * GROUP, (g + 1) * GROUP)
                nc.sync.dma_start(out=yt0[:, sl], in_=yt[0:P, sl])
                nc.sync.dma_start(out=yt1[:, sl], in_=yt[P : 2 * P, sl])

            for t in range(M_TILES):
                msl = slice(t * P, (t + 1) * P)
                for g in range(N_GROUPS):
                    ps = ppool.tile([P, GROUP], f32, tag="ps")
                    for b in range(GROUP // BANK):
                        nsl = slice(g * GROUP + b * BANK, g * GROUP + (b + 1) * BANK)
                        bsl = slice(b * BANK, (b + 1) * BANK)
                        nc.tensor.matmul(
                            ps[:, bsl], xt0[:, msl], yt0[:, nsl],
                            start=True, stop=False,
                        )
                        nc.tensor.matmul(
                            ps[:, bsl], xt1[:, msl], yt1[:, nsl],
                            start=False, stop=False,
                        )
                        # += 1 * (-||y_j||^2 / 2) (hi+lo): completes x.y - y2/2
                        nc.tensor.matmul(
                            ps[:, bsl], ones, y2sb[:, nsl],
                            start=False, stop=True,
                        )  # k=2: ones[2,128].T @ y2pair[2,512]
                    ot = opool.tile([P, GROUP], f32, tag="ot")
                    # exp(2g*(x.y - y2/2) - g*x2) = exp(-g*(x2 + y2 - 2 x.y))
                    nc.scalar.activation(
                        ot, ps, mybir.ActivationFunctionType.Exp,
                        bias=x2sb[:, t : t + 1], scale=2.0 * GAMMA,
                    )
                    nc.sync.dma_start(
                        out=out[msl, g * GROUP : (g + 1) * GROUP], in_=ot
                    )

    _split_sync_waits(nc)
    return nc


def kernel(x: np.ndarray, y: np.ndarray) -> np.ndarray:
    from concourse import bass_utils

    x = np.asarray(x, dtype=np.float32)
    y = np.asarray(y, dtype=np.float32)

    if "nc" not in _cache:
        _cache["nc"] = _build()
    nc = _cache["nc"]

    import ml_dtypes

    bf16 = ml_dtypes.bfloat16
    yt = np.ascontiguousarray(y.T.astype(bf16))  # [256, 8192]
    xt_full = x.T.astype(bf16)  # [256, 8192]
    x2 = np.sum(x * x, axis=1)  # [8192]
    y2neg = -0.5 * np.sum(y.astype(np.float64) * y.astype(np.float64), axis=1)
    y2hi = y2neg.astype(bf16)
    y2lo = (y2neg - y2hi.astype(np.float64)).astype(bf16)
    y2pair = np.ascontiguousarray(np.stack([y2hi, y2lo], axis=0))  # [2, 8192]

    in_maps = []
    for c in range(N_CORES):
        cols = slice(c * M_SHARD, (c + 1) * M_SHARD)
        x2c = (-GAMMA * x2[cols]).astype(np.float32)
        in_maps.append(
            {
                "xt": np.ascontiguousarray(xt_full[:, cols]),
                "yt": yt,
                "x2": np.ascontiguousarray(x2c.reshape(M_TILES, P).T),
                "y2": y2pair,
            }
        )

    res = bass_utils.run_bass_kernel_spmd(
        nc, in_maps, core_ids=list(range(N_CORES))
    )
    _cache["last_result"] = res
    return np.concatenate([res.results[c]["out"] for c in range(N_CORES)], axis=0)


# revision 13
# speedup vs baseline: 2.8608x; 1.4619x over previous
"""RBF (Gaussian) kernel Gram matrix on 8 Trainium2 NeuronCores.

out[i, j] = exp(-gamma * ||x_i - y_j||^2),  x, y: [8192, 256] fp32.

Strategy (per the data-parallel-over-rows sharding):
  - Rows of x are sharded across the 8 cores (1024 rows each); y replicated.
  - Each core computes its [1024, 8192] stripe:
      psum = x_shard @ y.T - 0.5*||y||^2   (the -0.5*||y||^2 row is folded in
                                            as a k=1 matmul with a ones vector)
      out  = exp(2*gamma * psum + bias)    (bias = -gamma*||x||^2 per-partition,
                                            applied by the ACT engine for free)
    which equals exp(-gamma*(||x||^2 + ||y||^2 - 2*x.y)) exactly.
  - Host prep: transposed copies of x/y (contraction dim on partitions) and the
    row norms. PE does the GEMM, ACT does the exp straight out of PSUM, DMA
    streams the 33.5MB/core result to DRAM. All stages pipelined by Tile.
"""

import numpy as np

GAMMA = 0.005
FULL_N = 8192
D = 256
N_CORES = 8
M_SHARD = FULL_N // N_CORES  # 1024 rows of x per core
P = 128
M_TILES = M_SHARD // P  # 8
GROUP = 2048  # columns of output produced per PSUM fill (4 banks)
BANK = 512  # fp32 columns per PSUM bank (one matmul's max free dim)
N_GROUPS = FULL_N // GROUP  # 4

_cache = {}


def _split_sync_waits(nc, maxw=1):
    """walrus codegen rejects instructions carrying more than ~2 sync waits
    ("Too many sync wait commands"). Tile can attach many (e.g. the tail
    drain waits on every semaphore; a matmul can wait on several DMA lanes).
    Hoist the excess onto wait-only EventSemaphore instructions inserted
    just before the offender on the same engine (engines execute their
    instructions in block order, so all waits still precede the op)."""
    import concourse.mybir as mybir

    n_new = 0
    for fn in nc.m.functions:
        for bb in fn.blocks:
            insts = bb.instructions
            if not any(
                i.sync_info is not None and len(i.sync_info.on_wait) > maxw
                for i in insts
            ):
                continue
            new = []
            for inst in insts:
                si = inst.sync_info
                if si is not None and len(si.on_wait) > maxw:
                    waits = list(si.on_wait)
                    for i in range(0, len(waits) - maxw, maxw):
                        ev = mybir.InstEventSemaphore(
                            name=f"wsplit_{n_new}", ins=[], outs=[]
                        )
                        n_new += 1
                        ev.engine = inst.engine
                        ev.sync_info = mybir.SyncInfo(
                            on_wait=waits[i : i + maxw], on_update=[]
                        )
                        new.append(ev)
                    si.on_wait = waits[len(waits) - maxw :]
                new.append(inst)
            bb.instructions = new


def _build():
    import concourse.bass as bass
    import concourse.mybir as mybir
    import concourse.tile as tile

    f32 = mybir.dt.float32
    f16 = mybir.dt.float16
    nc = bass.Bass("TRN2", target_bir_lowering=False, debug=False)
    xt = nc.dram_tensor("xt", [D, M_SHARD], f16, kind="ExternalInput").ap()
    yt = nc.dram_tensor("yt", [D, FULL_N], f16, kind="ExternalInput").ap()
    x2 = nc.dram_tensor("x2", [P, M_TILES], f32, kind="ExternalInput").ap()
    # exp(-gamma*||y||^2) replicated across partitions, fp32; multiplied in
    # by the (otherwise idle) DVE so the PE only runs the 2 k-tile matmuls.
    ey = nc.dram_tensor("ey", [P, FULL_N], f32, kind="ExternalInput").ap()
    out = nc.dram_tensor("out", [M_SHARD, FULL_N], f32, kind="ExternalOutput").ap()

    with tile.TileContext(nc) as tc:
        with (
            tc.tile_pool(name="const", bufs=1) as cpool,
            tc.tile_pool(name="outp", bufs=3) as opool,
            tc.tile_pool(name="psum", bufs=2, space="PSUM") as ppool,
        ):
            # Small inputs first so the first matmul only gates on its own
            # y.T chunk, not the whole 4MB load.
            xt0 = cpool.tile([P, M_SHARD], f16, tag="xt0")
            xt1 = cpool.tile([P, M_SHARD], f16, tag="xt1")
            nc.sync.dma_start(out=xt0, in_=xt[0:P, :])
            nc.sync.dma_start(out=xt1, in_=xt[P : 2 * P, :])
            x2sb = cpool.tile([P, M_TILES], f32, tag="x2")
            nc.sync.dma_start(out=x2sb, in_=x2)
            eyb = cpool.tile([P, FULL_N], f32, tag="eyb")
            for g in range(N_GROUPS):
                sl = slice(g * GROUP, (g + 1) * GROUP)
                nc.sync.dma_start(out=eyb[:, sl], in_=ey[:, sl])
            # y.T resident in SBUF: two k-tiles of [128, 8192] fp16.
            yt0 = cpool.tile([P, FULL_N], f16, tag="yt0")
            yt1 = cpool.tile([P, FULL_N], f16, tag="yt1")
            for g in range(N_GROUPS):
                sl = slice(g * GROUP, (g + 1) * GROUP)
                nc.sync.dma_start(out=yt0[:, sl], in_=yt[0:P, sl])
                nc.sync.dma_start(out=yt1[:, sl], in_=yt[P : 2 * P, sl])

            for t in range(M_TILES):
                msl = slice(t * P, (t + 1) * P)
                for g in range(N_GROUPS):
                    ps = ppool.tile([P, GROUP], f32, tag="ps")
                    # weight-stationary order: one LDWEIGHTS per k-tile per
                    # group; consecutive matmuls hit different PSUM banks so
                    # fill/drain overlap.
                    for d, (xtd, ytd) in enumerate(((xt0, yt0), (xt1, yt1))):
                        for b in range(GROUP // BANK):
                            nsl = slice(
                                g * GROUP + b * BANK, g * GROUP + (b + 1) * BANK
                            )
                            bsl = slice(b * BANK, (b + 1) * BANK)
                            nc.tensor.matmul(
                                ps[:, bsl], xtd[:, msl], ytd[:, nsl],
                                start=(d == 0), stop=(d == 1),
                            )
                    ot = opool.tile([P, GROUP], f32, tag="ot")
                    # exp(2g*x.y - g*x2) * exp(-g*y2) = exp(-g*||x-y||^2)
                    nc.scalar.activation(
                        ot, ps, mybir.ActivationFunctionType.Exp,
                        bias=x2sb[:, t : t + 1], scale=2.0 * GAMMA,
                    )
                    gsl = slice(g * GROUP, (g + 1) * GROUP)
                    nc.vector.tensor_mul(out=ot, in0=ot, in1=eyb[:, gsl])
                    nc.sync.dma_start(
                        out=out[msl, g * GROUP : (g + 1) * GROUP], in_=ot
                    )

    _split_sync_waits(nc)
    return nc


def kernel(x: np.ndarray, y: np.ndarray) -> np.ndarray:
    from concourse import bass_utils

    x = np.asarray(x, dtype=np.float32)
    y = np.asarray(y, dtype=np.float32)

    if "nc" not in _cache:
        _cache["nc"] = _build()
    nc = _cache["nc"]

    yt = np.ascontiguousarray(y.T.astype(np.float16))  # [256, 8192]
    xt_full = x.T.astype(np.float16)  # [256, 8192]
    x2 = np.sum(x * x, axis=1)  # [8192]
    y2 = np.sum(y.astype(np.float64) * y.astype(np.float64), axis=1)
    eyrow = np.ascontiguousarray(
        np.broadcast_to(
            np.exp(-GAMMA * y2).astype(np.float32).reshape(1, FULL_N), (P, FULL_N)
        )
    )

    in_maps = []
    for c in range(N_CORES):
        cols = slice(c * M_SHARD, (c + 1) * M_SHARD)
        x2c = (-GAMMA * x2[cols]).astype(np.float32)
        in_maps.append(
            {
                "xt": np.ascontiguousarray(xt_full[:, cols]),
                "yt": yt,
                "x2": np.ascontiguousarray(x2c.reshape(M_TILES, P).T),
                "ey": eyrow,
            }
        )

    res = bass_utils.run_bass_kernel_spmd(
        nc, in_maps, core_ids=list(range(N_CORES))
    )
    _cache["last_result"] = res
    return np.concatenate([res.results[c]["out"] for c in range(N_CORES)], axis=0)


# revision 14
# speedup vs baseline: 3.4818x; 1.2171x over previous
"""RBF (Gaussian) kernel Gram matrix on 8 Trainium2 NeuronCores.

out[i, j] = exp(-gamma * ||x_i - y_j||^2),  x, y: [8192, 256] fp32.

Strategy (per the data-parallel-over-rows sharding):
  - Rows of x are sharded across the 8 cores (1024 rows each); y replicated.
  - Each core computes its [1024, 8192] stripe:
      psum = x_shard @ y.T - 0.5*||y||^2   (the -0.5*||y||^2 row is folded in
                                            as a k=1 matmul with a ones vector)
      out  = exp(2*gamma * psum + bias)    (bias = -gamma*||x||^2 per-partition,
                                            applied by the ACT engine for free)
    which equals exp(-gamma*(||x||^2 + ||y||^2 - 2*x.y)) exactly.
  - Host prep: transposed copies of x/y (contraction dim on partitions) and the
    row norms. PE does the GEMM, ACT does the exp straight out of PSUM, DMA
    streams the 33.5MB/core result to DRAM. All stages pipelined by Tile.
"""

import numpy as np

GAMMA = 0.005
FULL_N = 8192
D = 256
N_CORES = 8
M_SHARD = FULL_N // N_CORES  # 1024 rows of x per core
P = 128
M_TILES = M_SHARD // P  # 8
GROUP = 2048  # columns of output produced per PSUM fill (4 banks)
BANK = 512  # fp32 columns per PSUM bank (one matmul's max free dim)
N_GROUPS = FULL_N // GROUP  # 4

_cache = {}


def _split_sync_waits(nc, maxw=1):
    """walrus codegen rejects instructions carrying more than ~2 sync waits
    ("Too many sync wait commands"). Tile can attach many (e.g. the tail
    drain waits on every semaphore; a matmul can wait on several DMA lanes).
    Hoist the excess onto wait-only EventSemaphore instructions inserted
    just before the offender on the same engine (engines execute their
    instructions in block order, so all waits still precede the op)."""
    import concourse.mybir as mybir

    n_new = 0
    for fn in nc.m.functions:
        for bb in fn.blocks:
            insts = bb.instructions
            if not any(
                i.sync_info is not None and len(i.sync_info.on_wait) > maxw
                for i in insts
            ):
                continue
            new = []
            for inst in insts:
                si = inst.sync_info
                if si is not None and len(si.on_wait) > maxw:
                    waits = list(si.on_wait)
                    for i in range(0, len(waits) - maxw, maxw):
                        ev = mybir.InstEventSemaphore(
                            name=f"wsplit_{n_new}", ins=[], outs=[]
                        )
                        n_new += 1
                        ev.engine = inst.engine
                        ev.sync_info = mybir.SyncInfo(
                            on_wait=waits[i : i + maxw], on_update=[]
                        )
                        new.append(ev)
                    si.on_wait = waits[len(waits) - maxw :]
                new.append(inst)
            bb.instructions = new


def _build():
    import concourse.bass as bass
    import concourse.mybir as mybir
    import concourse.tile as tile

    f32 = mybir.dt.float32
    f16 = mybir.dt.float16
    nc = bass.Bass("TRN2", target_bir_lowering=False, debug=False)
    xt = nc.dram_tensor("xt", [D, M_SHARD], f16, kind="ExternalInput").ap()
    yt = nc.dram_tensor("yt", [D, FULL_N], f16, kind="ExternalInput").ap()
    x2 = nc.dram_tensor("x2", [P, M_TILES], f32, kind="ExternalInput").ap()
    # -||y||^2/2 as an fp16 hi/lo pair (row 1 = residual): folded into the
    # Gram matmul as a k=2 ones-matmul, losing only ~2^-21 relative error.
    y2 = nc.dram_tensor("y2", [2, FULL_N], f16, kind="ExternalInput").ap()
    out = nc.dram_tensor("out", [M_SHARD, FULL_N], f32, kind="ExternalOutput").ap()

    with tile.TileContext(nc) as tc:
        with (
            tc.tile_pool(name="const", bufs=1) as cpool,
            tc.tile_pool(name="outp", bufs=3) as opool,
            tc.tile_pool(name="psum", bufs=2, space="PSUM") as ppool,
        ):
            # Small inputs first so the first matmul only gates on its own
            # y.T chunk, not the whole 4MB load.
            xt0 = cpool.tile([P, M_SHARD], f16, tag="xt0")
            xt1 = cpool.tile([P, M_SHARD], f16, tag="xt1")
            nc.sync.dma_start(out=xt0, in_=xt[0:P, :])
            nc.sync.dma_start(out=xt1, in_=xt[P : 2 * P, :])
            x2sb = cpool.tile([P, M_TILES], f32, tag="x2")
            nc.sync.dma_start(out=x2sb, in_=x2)
            y2sb = cpool.tile([2, FULL_N], f16, tag="y2")
            nc.sync.dma_start(out=y2sb, in_=y2)
            ones = cpool.tile([2, P], f16, tag="ones")
            nc.any.memset(ones, 1.0)
            # y.T resident in SBUF: two k-tiles of [128, 8192] fp16.
            yt0 = cpool.tile([P, FULL_N], f16, tag="yt0")
            yt1 = cpool.tile([P, FULL_N], f16, tag="yt1")
            for g in range(N_GROUPS):
                sl = slice(g * GROUP, (g + 1) * GROUP)
                nc.sync.dma_start(out=yt0[:, sl], in_=yt[0:P, sl])
                nc.sync.dma_start(out=yt1[:, sl], in_=yt[P : 2 * P, sl])

            for t in range(M_TILES):
                msl = slice(t * P, (t + 1) * P)
                for g in range(N_GROUPS):
                    ps = ppool.tile([P, GROUP], f32, tag="ps")
                    # weight-stationary order: one LDWEIGHTS per k-tile per
                    # group; consecutive matmuls hit different PSUM banks so
                    # fill/drain overlap.
                    for d, (lhs_full, ytd) in enumerate(
                        ((xt0, yt0), (xt1, yt1), (ones, y2sb))
                    ):
                        lhs = lhs_full if d == 2 else lhs_full[:, msl]
                        for b in range(GROUP // BANK):
                            nsl = slice(
                                g * GROUP + b * BANK, g * GROUP + (b + 1) * BANK
                            )
                            bsl = slice(b * BANK, (b + 1) * BANK)
                            nc.tensor.matmul(
                                ps[:, bsl], lhs, ytd[:, nsl],
                                start=(d == 0), stop=(d == 2),
                            )
                    ot = opool.tile([P, GROUP], f32, tag="ot")
                    # exp(2g*(x.y - y2/2) - g*x2) = exp(-g*||x-y||^2)
                    nc.scalar.activation(
                        ot, ps, mybir.ActivationFunctionType.Exp,
                        bias=x2sb[:, t : t + 1], scale=2.0 * GAMMA,
                    )
                    nc.sync.dma_start(
                        out=out[msl, g * GROUP : (g + 1) * GROUP], in_=ot
                    )

    _split_sync_waits(nc)
    return nc


def kernel(x: np.ndarray, y: np.ndarray) -> np.ndarray:
    from concourse import bass_utils

    x = np.asarray(x, dtype=np.float32)
    y = np.asarray(y, dtype=np.float32)

    if "nc" not in _cache:
        _cache["nc"] = _build()
    nc = _cache["nc"]

    yt = np.ascontiguousarray(y.T.astype(np.float16))  # [256, 8192]
    xt_full = x.T.astype(np.float16)  # [256, 8192]
    x2 = np.sum(x * x, axis=1)  # [8192]
    y2neg = -0.5 * np.sum(y.astype(np.float64) * y.astype(np.float64), axis=1)
    y2hi = y2neg.astype(np.float16)
    y2lo = (y2neg - y2hi.astype(np.float64)).astype(np.float16)
    y2pair = np.ascontiguousarray(np.stack([y2hi, y2lo], axis=0))  # [2, 8192]

    in_maps = []
    for c in range(N_CORES):
        cols = slice(c * M_SHARD, (c + 1) * M_SHARD)
        x2c = (-GAMMA * x2[cols]).astype(np.float32)
        in_maps.append(
            {
                "xt": np.ascontiguousarray(xt_full[:, cols]),
                "yt": yt,
                "x2": np.ascontiguousarray(x2c.reshape(M_TILES, P).T),
                "y2": y2pair,
            }
        )

    res = bass_utils.run_bass_kernel_spmd(
        nc, in_maps, core_ids=list(range(N_CORES))
    )
    _cache["last_result"] = res
    return np.concatenate([res.results[c]["out"] for c in range(N_CORES)], axis=0)
